# revision 5
# baseline (speedup 1.0000x reference)
"""CRF-RNN mean-field kernel for Trainium2 (8 NeuronCores, data-parallel over T).

Math: reference computes, with x0 = inputs @ W_feat.T (T,N),
A[i,j] = sum_k kernels[i,j,k] W_lin[k], denom[i] = sum(W_feat) + 2*sum_j A[i,j],
the 4-step recurrence  x <- (x0 + 2 x A^T) / denom.
The recurrence is linear, so with D = diag(1/denom), B = 2 A^T D:
    x4 = x0 @ E,   E = D (I + B + B^2 + B^3) + B^4     (256x256, precomputed on-chip)

v2 layout: T sharded 8 ways (2048 rows/core); every core loads the full 4MB
kernels (two 2MB halves on the two load rings) and builds A/E locally — no
collective. x0 is computed as 4 pair-products on the DVE (custom op
out = Src0*c0 + Src1*c1 reading two streams/cycle; weights are per-pair
scalars) yielding 4 partial x0^p; the partials are summed for free on the
TensorEngine: the x0 transpose is 4 accumulating matmuls against identity.
Stream: 8 x 2MB input blocks, loads alternating sync/gpsimd rings, stores on
the scalar ring; PSUM->SBUF copies split between ACT and DVE.
"""

import os
import sys

for _p in ("/opt/trn_rl_repo",):
    if _p not in sys.path and os.path.isdir(_p):
        sys.path.insert(0, _p)

import numpy as np

import concourse.bass as bass
import concourse.mybir as mybir
from concourse import bacc
from concourse.bass_utils import run_bass_kernel_spmd
from concourse.masks import make_identity
from concourse.tile import TileContext

F32 = mybir.dt.float32
AL = mybir.AluOpType
AX = mybir.AxisListType


def _register_pairmac():
    """Custom DVE op: out = Src0*C0 + Src1*C1 (two full tensor streams, two
    per-partition scalars). Both SBUF read ports carry data, so one pair of
    weighted input lanes is consumed per cycle."""
    import concourse.dve_ops as dve_ops
    from concourse.dve_ops import DveOp
    from concourse.dve_spec import C0, C1, Spec, Src0, Src1, lower
    from concourse.dve_uop import DveOpSpec

    if hasattr(dve_ops, "TENSOR_PAIRMAC_ANT"):
        return dve_ops.TENSOR_PAIRMAC_ANT

    def ref(in0, in1, s0, s1, imm2):
        a = np.asarray(in0, np.float32)
        b = np.asarray(in1, np.float32).reshape(a.shape)
        c0 = np.asarray(s0, np.float32).reshape(-1, *([1] * (a.ndim - 1)))
        c1 = np.asarray(s1, np.float32).reshape(-1, *([1] * (a.ndim - 1)))
        return a * c0 + b * c1

    name = "TENSOR_PAIRMAC_ANT"
    spec = Spec(body=Src0 * C0 + Src1 * C1, reference=ref)
    row = max(dve_ops._SUB_OPCODE_FOR_NAME.values()) + 1
    assert row < 0x20, "custom-DVE opcode rows exhausted"
    shas = {}
    for ver in ("v3", "v4"):
        try:
            shas[ver] = DveOpSpec(
                name=name, opcode=row, uops=lower(spec, ver=ver), rd1_en=True
            ).sha(ver)
        except Exception:
            pass
    op = DveOp(name, spec, subdim=False, uops_sha=shas)
    dve_ops.OPS.append(op)
    dve_ops._SUB_OPCODE_FOR_NAME[op.name] = row
    dve_ops.CUSTOM_DVE_SPECS[op.name] = op.spec
    dve_ops.TENSOR_PAIRMAC_ANT = op
    return op


T, N, M, K = 16384, 256, 8, 16
NCORES = 8
TL = T // NCORES  # 2048 rows per core
P = 128
NH = N // P  # 2 region halves
BL = 2 * P  # t-rows per DMA block (2MB loads)
NB = TL // BL  # 8 blocks per core
NQ = BL // P  # 2 psum-sized subtiles per block
SCN = NQ * N * M  # 4096 input elems per partition per block
NG = NQ * N  # 512 x0 groups per partition per block


def _strided(apview, offset, stride, size):
    """Return a [P, size] view of `apview` starting at elem `offset` with
    free-dim stride `stride` (in elements)."""
    import dataclasses

    base = apview[:, offset : offset + 1]
    return dataclasses.replace(base, ap=[base.ap[0], [stride, size]])


def _kernel_body(tc, inp, kern, wf, wl, out, mode="full", stream_loop_cm=None):
    """mode: 'full' | 'dma' (loads+stores only) | 'x0' (dma + DVE pairmacs)
    | 'pe' (dma + transpose/matmul path, x0 faked from raw input)."""
    nc = tc.nc
    pairmac = _register_pairmac()

    import contextlib
    from contextlib import ExitStack

    with ExitStack() as ctx:
        const = ctx.enter_context(tc.tile_pool(name="const", bufs=1))
        kernp = ctx.enter_context(tc.tile_pool(name="kernp", bufs=2))
        prep = ctx.enter_context(tc.tile_pool(name="prep", bufs=1))
        work = ctx.enter_context(tc.tile_pool(name="work", bufs=4))
        x0p = ctx.enter_context(tc.tile_pool(name="x0p", bufs=3))
        xsp = ctx.enter_context(tc.tile_pool(name="xsp", bufs=4))
        outp = ctx.enter_context(tc.tile_pool(name="outp", bufs=3))
        pst = ctx.enter_context(tc.tile_pool(name="pst", bufs=4, space="PSUM"))
        pso = ctx.enter_context(tc.tile_pool(name="pso", bufs=3, space="PSUM"))
        psw = ctx.enter_context(tc.tile_pool(name="psw", bufs=1, space="PSUM"))

        # ---------------- constants ----------------
        ident = const.tile([P, P], F32)
        make_identity(nc, ident[:])

        wf_row = const.tile([1, M], F32)
        nc.sync.dma_start(wf_row[:], wf[:, :])
        wf_sb = const.tile([P, M], F32)
        nc.gpsimd.partition_broadcast(wf_sb[:], wf_row[:])

        wl_row = const.tile([1, K], F32)
        nc.sync.dma_start(wl_row[:], wl[:, :])
        wl_sb = const.tile([P, K], F32)
        nc.gpsimd.partition_broadcast(wl_sb[:], wl_row[:])

        fw_sum = const.tile([P, 1], F32)
        nc.vector.tensor_reduce(fw_sum[:], wf_sb[:], axis=AX.X, op=AL.add)

        # ------------- precompute E (every core, identical) -------------
        E = [const.tile([P, N], F32, tag=f"E{jb}", name=f"E{jb}") for jb in range(NH)]
        if mode != "full":
            kt0 = kernp.tile([P, N * K], F32, tag="kern", name="kern0")
            nc.sync.dma_start(kt0[:], kern[0:P, :])
            kt1 = kernp.tile([P, N * K], F32, tag="kern", name="kern1")
            nc.gpsimd.dma_start(kt1[:], kern[P : 2 * P, :])
            if mode == "pe":
                for jb in range(NH):
                    nc.gpsimd.memset(E[jb][:], 0.001)
        else:
            # A[i,j] = sum_k kern[i,j,k] * wl[k] via 8 pair-products + tree add
            Bt = []  # Bt[h][i_loc, j] = B[j, h*128+i_loc] = 2*invd[i]*A[i,j]
            invd = []  # [128,1] per half, partition index = region index
            for h in range(NH):
                kt = kernp.tile([P, N * K], F32, tag="kern", name=f"kern{h}")
                eng = nc.sync if h == 0 else nc.gpsimd
                eng.dma_start(kt[:], kern[h * P : (h + 1) * P, :])
                apart = prep.tile([P, 8 * N], F32, tag="apart", name=f"apart{h}")
                for p in range(8):
                    nc.vector._custom_dve(
                        pairmac,
                        out=apart[:, p * N : (p + 1) * N],
                        in0=_strided(kt[:], 2 * p, K, N),
                        in1=_strided(kt[:], 2 * p + 1, K, N),
                        s0=wl_sb[:, 2 * p : 2 * p + 1],
                        s1=wl_sb[:, 2 * p + 1 : 2 * p + 2],
                    )
                a4 = prep.tile([P, 4 * N], F32, tag="a4", name=f"a4_{h}")
                nc.vector.tensor_add(a4[:], apart[:, : 4 * N], apart[:, 4 * N :])
                a2 = prep.tile([P, 2 * N], F32, tag="a2", name=f"a2_{h}")
                nc.vector.tensor_add(a2[:], a4[:, : 2 * N], a4[:, 2 * N :])
                Ah = const.tile([P, N], F32, tag=f"A{h}", name=f"A{h}")
                nc.vector.tensor_add(Ah[:], a2[:, :N], a2[:, N:])

                red = const.tile([P, 1], F32, tag=f"red{h}", name=f"red{h}")
                nc.vector.tensor_reduce(red[:], Ah[:], axis=AX.X, op=AL.add)
                den = const.tile([P, 1], F32, tag=f"den{h}", name=f"den{h}")
                nc.vector.scalar_tensor_tensor(
                    den[:], red[:], 2.0, fw_sum[:], op0=AL.mult, op1=AL.add
                )
                inv = const.tile([P, 1], F32, tag=f"invd{h}", name=f"invd{h}")
                nc.vector.reciprocal(inv[:], den[:])
                invd.append(inv)
                inv2 = const.tile([P, 1], F32, tag=f"invd2{h}", name=f"invd2{h}")
                nc.vector.tensor_scalar_mul(inv2[:], inv[:], 2.0)
                Bth = const.tile([P, N], F32, tag=f"Bt{h}", name=f"Bt{h}")
                nc.scalar.mul(Bth[:], Ah[:], inv2[:, 0:1])
                Bt.append(Bth)

            # B1[jb][j_loc, i] = B[jb*128+j_loc, i]  (PE transpose of Bt blocks)
            B1 = [
                const.tile([P, N], F32, tag=f"B1{jb}", name=f"B1{jb}")
                for jb in range(NH)
            ]
            for jb in range(NH):
                for ih in range(NH):
                    pt = pst.tile([P, P], F32, tag="tr", name=f"trB{jb}{ih}")
                    nc.tensor.transpose(
                        pt[:], Bt[ih][:, jb * P : (jb + 1) * P], ident[:]
                    )
                    nc.scalar.copy(B1[jb][:, ih * P : (ih + 1) * P], pt[:])

            # Powers: B_{n+1}[j,i] = sum_l Bt[l,j] * B_n[l,i]
            def mat_next(rhs_tiles, tag):
                res = [
                    const.tile([P, N], F32, tag=f"{tag}{jb}", name=f"{tag}{jb}")
                    for jb in range(NH)
                ]
                for jb in range(NH):
                    ps = psw.tile([P, N], F32, tag="pw", name=f"pw{tag}{jb}")
                    for lh in range(NH):
                        nc.tensor.matmul(
                            ps[:],
                            Bt[lh][:, jb * P : (jb + 1) * P],
                            rhs_tiles[lh][:],
                            start=(lh == 0),
                            stop=(lh == NH - 1),
                        )
                    nc.scalar.copy(res[jb][:], ps[:])
                return res

            B2 = mat_next(B1, "B2")
            B3 = mat_next(B2, "B3")
            B4 = mat_next(B3, "B4")

            # E[jb] = invd ⊙ (I + B1 + B2 + B3)[jb] + B4[jb]
            for jb in range(NH):
                s = E[jb]
                nc.vector.tensor_add(s[:], B1[jb][:], B2[jb][:])
                nc.vector.tensor_add(s[:], s[:], B3[jb][:])
                nc.vector.tensor_add(
                    s[:, jb * P : (jb + 1) * P], s[:, jb * P : (jb + 1) * P], ident[:]
                )
                nc.scalar.mul(s[:], s[:], invd[jb][:, 0:1])
                nc.vector.tensor_add(s[:], s[:], B4[jb][:])

        # ------------- main stream: 8 blocks of 256 t-rows -------------
        stream_cm = stream_loop_cm() if stream_loop_cm else contextlib.nullcontext()
        ctx.enter_context(stream_cm)
        for b in range(NB):
            it2 = work.tile([P, SCN], F32, tag="in", name=f"in{b}")
            src = inp[b * BL : (b + 1) * BL, :].rearrange("(q p) f -> p q f", p=P)
            ld_eng = nc.sync if b % 2 == 0 else nc.gpsimd
            ld_eng.dma_start(it2[:].rearrange("p (q f) -> p q f", q=NQ), src)

            if mode in ("full", "x0"):
                # x0^p[t, g] = in[t, 8g+2p]*wf[2p] + in[t, 8g+2p+1]*wf[2p+1]
                x0b = x0p.tile([P, 4 * NG], F32, tag="x0", name=f"x0{b}")
                for p in range(4):
                    nc.vector._custom_dve(
                        pairmac,
                        out=x0b[:, p * NG : (p + 1) * NG],
                        in0=_strided(it2[:], 2 * p, M, NG),
                        in1=_strided(it2[:], 2 * p + 1, M, NG),
                        s0=wf_sb[:, 2 * p : 2 * p + 1],
                        s1=wf_sb[:, 2 * p + 1 : 2 * p + 2],
                    )
            elif mode == "pe":
                x0b = it2[:, 0 : 4 * NG]
            else:
                x0b = None

            if mode in ("full", "pe"):
                ot2 = outp.tile([P, NQ * N], F32, tag="ot", name=f"ot{b}")
                for q in range(NQ):
                    x0T = []
                    for jb in range(NH):
                        # transpose-with-accumulate: x0T = sum_p (x0^p slice)^T
                        pt = pst.tile([P, P], F32, tag="tr", name=f"tr{b}_{q}{jb}")
                        for p in range(4):
                            nc.tensor.matmul(
                                pt[:],
                                x0b[
                                    :,
                                    p * NG + q * N + jb * P : p * NG
                                    + q * N
                                    + (jb + 1) * P,
                                ],
                                ident[:],
                                start=(p == 0),
                                stop=(p == 3),
                            )
                        xs = xsp.tile(
                            [P, P], F32, tag=f"x0T{jb}", name=f"x0T{b}_{q}{jb}"
                        )
                        if jb == 0:
                            nc.scalar.copy(xs[:], pt[:])
                        else:
                            nc.vector.tensor_copy(xs[:], pt[:])
                        x0T.append(xs)
                    po = pso.tile([P, N], F32, tag="out", name=f"po{b}_{q}")
                    for jb in range(NH):
                        nc.tensor.matmul(
                            po[:],
                            x0T[jb][:],
                            E[jb][:],
                            start=(jb == 0),
                            stop=(jb == NH - 1),
                        )
                    nc.scalar.copy(ot2[:, q * N : (q + 1) * N], po[:])
            elif mode == "x0":
                ot2 = x0b[:, 0 : NQ * N]
            else:  # dma
                ot2 = outp.tile([P, NQ * N], F32, tag="ot", name=f"ot{b}")
                nc.gpsimd.memset(ot2[:], 0.0)
            dst = out[b * BL : (b + 1) * BL, :].rearrange("(q p) i -> p q i", p=P)
            nc.scalar.dma_start(dst, ot2[:].rearrange("p (q i) -> p q i", q=NQ))


_NC_CACHE = {}


def _build(bodies=1):
    if bodies in _NC_CACHE:
        return _NC_CACHE[bodies]
    nc = bacc.Bacc(
        "TRN2",
        target_bir_lowering=False,
        debug=False,
        enable_asserts=False,
        num_devices=NCORES,
    )
    inp = nc.dram_tensor("inp", (TL, N * M), F32, kind="ExternalInput").ap()
    kern = nc.dram_tensor("kern", (N, N * K), F32, kind="ExternalInput").ap()
    wf = nc.dram_tensor("wf", (1, M), F32, kind="ExternalInput").ap()
    wl = nc.dram_tensor("wl", (1, K), F32, kind="ExternalInput").ap()
    out = nc.dram_tensor("out", (TL, N), F32, kind="ExternalOutput").ap()
    with TileContext(nc) as tc:
        for _ in range(bodies):
            _kernel_body(tc, inp, kern, wf, wl, out)
    nc.compile()
    _NC_CACHE[bodies] = nc
    return nc


def _build_loop(mode="full"):
    """Variant with the stream inside a dynamic For_i whose bound comes from
    the int32 input `reps` — one executable, runtime body count, for timing."""
    key = ("loop", mode)
    if key in _NC_CACHE:
        return _NC_CACHE[key]
    nc = bacc.Bacc(
        "TRN2",
        target_bir_lowering=False,
        debug=False,
        enable_asserts=False,
        num_devices=NCORES,
    )
    inp = nc.dram_tensor("inp", (TL, N * M), F32, kind="ExternalInput").ap()
    kern = nc.dram_tensor("kern", (N, N * K), F32, kind="ExternalInput").ap()
    wf = nc.dram_tensor("wf", (1, M), F32, kind="ExternalInput").ap()
    wl = nc.dram_tensor("wl", (1, K), F32, kind="ExternalInput").ap()
    reps = nc.dram_tensor("reps", (1, 1), mybir.dt.int32, kind="ExternalInput").ap()
    out = nc.dram_tensor("out", (TL, N), F32, kind="ExternalOutput").ap()
    with TileContext(nc) as tc:
        with tc.tile_pool(name="repsp", bufs=1) as rp:
            reps_sb = rp.tile([1, 1], mybir.dt.int32)
            nc.sync.dma_start(reps_sb[:], reps[:, :])
            r_val = nc.values_load(
                reps_sb[:], min_val=0, max_val=256, skip_runtime_bounds_check=True
            )
            _kernel_body(
                tc, inp, kern, wf, wl, out, mode=mode,
                stream_loop_cm=lambda: tc.For_i(0, r_val, 1),
            )
    nc.compile()
    _NC_CACHE[key] = nc
    return nc


def bench_loop(rvals=(1, 501), reps=16, mode="full"):
    """Time one executable at different runtime body counts R; per-dispatch
    offsets cancel in the R-slope."""
    import time

    import jax

    rng = np.random.default_rng(0)
    inp = rng.standard_normal((T, N * M), dtype=np.float32)
    kr = rng.random((N, N * K), dtype=np.float32)
    wf = (rng.random((1, M), dtype=np.float32) * 0.01).astype(np.float32)
    wl = (rng.random((1, K), dtype=np.float32) * 0.01).astype(np.float32)

    nc = _build_loop(mode)
    fn, in_names, out_names, out_avals, sh = _pjrt_callable(nc)
    argsets = {}
    for rv in rvals:
        cat = {
            "inp": inp,
            "kern": np.tile(kr, (NCORES, 1)),
            "wf": np.tile(wf, (NCORES, 1)),
            "wl": np.tile(wl, (NCORES, 1)),
            "reps": np.full((NCORES, 1), rv, np.int32),
        }
        args = [jax.device_put(cat[n], sh) for n in in_names]
        args += [
            jax.device_put(np.zeros((NCORES * a.shape[0], *a.shape[1:]), a.dtype), sh)
            for a in out_avals
        ]
        o = fn(*args)
        np.asarray(o[0])  # warm; forced sync via value fetch
        argsets[rv] = args
    rlo, rhi = min(rvals), max(rvals)
    slopes = []
    for _ in range(reps):
        t0 = time.perf_counter()
        o = fn(*argsets[rlo])
        np.asarray(o[0])
        tl = time.perf_counter() - t0
        t0 = time.perf_counter()
        o = fn(*argsets[rhi])
        np.asarray(o[0])
        th = time.perf_counter() - t0
        slopes.append((th - tl) / (rhi - rlo) * 1e9)
    slopes.sort()
    slope_ns = slopes[len(slopes) // 2]
    print(
        f"paired slope (R={rhi} vs R={rlo}, {reps} pairs): median {slope_ns:.0f} ns "
        f"(p25 {slopes[len(slopes)//4]:.0f}, p75 {slopes[3*len(slopes)//4]:.0f})"
    )
    return slope_ns, slopes


def kernel(inputs, kernels, W_feat, W_lin, trace=False):
    inp = np.ascontiguousarray(np.asarray(inputs, dtype=np.float32).reshape(T, N * M))
    kr = np.ascontiguousarray(np.asarray(kernels, dtype=np.float32).reshape(N, N * K))
    wf = np.ascontiguousarray(np.asarray(W_feat, dtype=np.float32).reshape(1, M))
    wl = np.ascontiguousarray(np.asarray(W_lin, dtype=np.float32).reshape(1, K))

    nc = _build(1)
    in_maps = [
        {
            "inp": inp[c * TL : (c + 1) * TL],
            "kern": kr,
            "wf": wf,
            "wl": wl,
        }
        for c in range(NCORES)
    ]
    res = run_bass_kernel_spmd(nc, in_maps, core_ids=list(range(NCORES)), trace=trace)
    outs = [res.results[c]["out"] for c in range(NCORES)]
    full = np.concatenate(outs, axis=0).reshape(T, N, 1)
    if trace:
        kernel.last_exec_time_ns = res.exec_time_ns
        kernel.last_results = res
    return full


def _pjrt_callable(nc):
    """Build a jit(shard_map(bass_exec)) callable + device-resident input list,
    mirroring bass2jax.run_bass_via_pjrt (no donation: outputs reallocated)."""
    import jax
    from jax.sharding import Mesh, NamedSharding, PartitionSpec
    from jax.experimental.shard_map import shard_map

    from concourse.bass2jax import (
        _bass_exec_p,
        install_neuronx_cc_hook,
        partition_id_tensor,
    )

    install_neuronx_cc_hook()
    partition_name = nc.partition_id_tensor.name if nc.partition_id_tensor else None
    in_names, out_names, out_avals = [], [], []
    for alloc in nc.m.functions[0].allocations:
        if not isinstance(alloc, mybir.MemoryLocationSet):
            continue
        name = alloc.memorylocations[0].name
        if alloc.kind == "ExternalInput":
            if name != partition_name:
                in_names.append(name)
        elif alloc.kind == "ExternalOutput":
            out_names.append(name)
            out_avals.append(
                jax.core.ShapedArray(tuple(alloc.tensor_shape), mybir.dt.np(alloc.dtype))
            )
    all_in = list(in_names) + list(out_names)
    if partition_name is not None:
        all_in.append(partition_name)
    all_in = tuple(all_in)

    def _body(*args):
        operands = list(args)
        if partition_name is not None:
            operands.append(partition_id_tensor())
        return tuple(
            _bass_exec_p.bind(
                *operands,
                out_avals=tuple(out_avals),
                in_names=all_in,
                out_names=tuple(out_names),
                lowering_input_output_aliases=(),
                sim_require_finite=True,
                sim_require_nnan=True,
                nc=nc,
            )
        )

    devices = jax.devices()[:NCORES]
    mesh = Mesh(np.asarray(devices), ("core",))
    nin = len(in_names) + len(out_names)
    fn = jax.jit(
        shard_map(
            _body,
            mesh=mesh,
            in_specs=(PartitionSpec("core"),) * nin,
            out_specs=(PartitionSpec("core"),) * len(out_names),
            check_rep=False,
        )
    )
    sh = NamedSharding(mesh, PartitionSpec("core"))
    return fn, in_names, out_names, out_avals, sh
